# revision 2
# baseline (speedup 1.0000x reference)
"""MoE grouped-GEMM expert MLP for Trainium2, expert-parallel over 8 NeuronCores.

Problem: x:(B=2, E=8, N=2048, D=1024), per-expert 2-layer GELU MLP with
w1:(E, D, F=4096), w2:(E, F, D).  Reference computes
  xe = x.reshape(E, B*N, D)          # pure buffer reinterpretation
  h  = gelu_tanh(xe @ w1 + b1)
  out= h @ w2 + b2                   # reshaped back to (B, E, N, D)

Sharding: expert parallelism - core e runs expert e on its contiguous
token block xe[e] (4096 tokens).  No collectives needed.

Per-core layout: hidden activations kept transposed ("hT" = [f, tok]) so
both weight matrices are consumed in their NATIVE layouts:
  GEMM1: hT[f,tok]  = (w1[d,f] as lhsT).T @ xT[d,tok]
  GEMM2: out[tok,d] = (hT[f,tok] slice as lhsT).T @ w2[f,d]

v2 (startup redesign; v1 measured 986us with ~87us PE idle + ~24us
HAM cold-clock penalty, all in the first 200us while weights streamed):
 - SWDGE (gpsimd) queues carry ONLY the w1/w2 streams, in consumption
   order (w1 g0 split fine-grained so GEMM1 starts ~7us).
 - x path entirely off SWDGE: fp32 quarters via sync HWDGE -> DVE cast
   to bf16 -> XBAR SBUF->SBUF DMA transpose.  No DRAM scratch.
 - GEMM2 is fo-outer with all 8 PSUM banks as whole-chunk accumulators,
   so w2 groups are consumed in streaming order (group k needed at
   ~6.8k us after GEMM2 starts vs delivery ~5.9k us).
 - PE pre-warm: junk matmuls at t=0 so HAM un-throttles (2.4GHz) before
   real work, and PE never idles >3.4us (no re-throttle).
 - outputs on the scalar HWDGE queue (sync queue carries x traffic).

Compute dtype bf16 (fp32 PSUM accumulation), gelu on ScalarE matching
jax.nn.gelu(approximate=True): end-to-end rel-err ~3.4e-3.
"""

import numpy as np

import concourse.bacc as bacc
import concourse.mybir as mybir
import concourse.tile as tile
from concourse.bass_utils import run_bass_kernel_spmd

E, B, N, D, F = 8, 2, 2048, 1024, 4096
TOK = B * N            # tokens per expert / per core
TC = 512               # token chunk processed per pipeline stage
NCHUNK = TOK // TC     # 8
P = 128
DO = D // P            # 8  d-tiles (GEMM1 contraction)
FO = F // P            # 32 f-tiles (GEMM2 contraction)
FG = 8                 # weight f-groups of 512 (4 f-tiles each)
NPREWARM = 26          # junk matmuls covering PE until real work lands

F32 = mybir.dt.float32
BF16 = mybir.dt.bfloat16
GELU = mybir.ActivationFunctionType.Gelu_apprx_tanh


def _build_kernel(tc_ctx, nc, x, w1, b1, w2, b2, out):
    with (
        tc_ctx.tile_pool(name="wpool", bufs=1) as wp,
        tc_ctx.tile_pool(name="xinp", bufs=2) as xinp,
        tc_ctx.tile_pool(name="xcp", bufs=1) as xcp,
        tc_ctx.tile_pool(name="xtp", bufs=2) as xtp,
        tc_ctx.tile_pool(name="hpool", bufs=1) as hp,
        tc_ctx.tile_pool(name="opool", bufs=3) as op,
        tc_ctx.tile_pool(name="cpool", bufs=1) as cp,
        tc_ctx.tile_pool(name="ps", bufs=8, space="PSUM") as psp,
    ):
        # ---- PE pre-warm: HAM un-throttles after ~3.4us of sustained
        # matmul activity; cover t=0..~7us so GEMM1-c0 runs at 2.4GHz.
        junkw = cp.tile([P, P], BF16, tag="junkw")
        junkx = cp.tile([P, TC], BF16, tag="junkx")
        nc.vector.memset(junkw, 0.0)
        nc.vector.memset(junkx, 0.0)
        pw = psp.tile([P, TC], F32, tag="ps", name="prewarm")
        for i in range(NPREWARM):
            nc.tensor.matmul(
                pw, junkw, junkx, start=(i == 0), stop=(i == NPREWARM - 1)
            )

        # ---- biases on the scalar HWDGE queue (idle otherwise) ----
        b1sb = cp.tile([P, FO], F32, tag="b1")
        nc.scalar.dma_start(b1sb, b1.rearrange("(fo fi) -> fi fo", fi=P))
        b2sb = cp.tile([P, D], F32, tag="b2")
        nc.scalar.dma_start(b2sb, b2[None, :].partition_broadcast(P))

        # ---- weights: SWDGE only, in consumption order ----
        # w1 tile (ki, do, fj) = w1[do*128+ki, fg*512+fj] : lhsT for GEMM1
        w1r = w1.rearrange("(do ki) f -> ki do f", ki=P)
        # w2 tile (ki, m, dj) = w2[fg*512 + m*128 + ki, dj] : rhs for GEMM2
        w2r = w2.rearrange("(fg m ki) d -> ki fg m d", ki=P, m=4)
        w1g = [
            wp.tile([P, DO, 512], BF16, tag=f"w1g{fg}", name=f"w1g{fg}")
            for fg in range(FG)
        ]
        w2g = [
            wp.tile([P, 4, D], BF16, tag=f"w2g{fg}", name=f"w2g{fg}")
            for fg in range(FG)
        ]
        # first w1 group split per do-pair so GEMM1-c0-fo0 starts ASAP
        for dp in range(4):
            nc.gpsimd.dma_start(
                w1g[0][:, 2 * dp:2 * dp + 2, :], w1r[:, 2 * dp:2 * dp + 2, 0:512]
            )
        for fg in range(1, FG):
            nc.gpsimd.dma_start(w1g[fg], w1r[:, :, fg * 512:(fg + 1) * 512])
        for fg in range(FG):
            nc.gpsimd.dma_start(w2g[fg], w2r[:, fg])

        # ---- x prep: fp32 quarter loads (sync HWDGE) -> DVE bf16 cast
        # -> XBAR SBUF->SBUF transpose (sync HWDGE) ----
        xT = [None] * NCHUNK

        def emit_xprep(c):
            xc = xcp.tile([P, 4, D], BF16, tag="xc", name=f"xc{c}")
            xt = xtp.tile([P, DO, TC], BF16, tag="xT", name=f"xT{c}")
            xqs = []
            for tm in range(4):
                xq = xinp.tile([P, D], F32, tag="xq", name=f"xq{c}_{tm}")
                nc.sync.dma_start(
                    xq, x[c * TC + tm * P:c * TC + (tm + 1) * P, :]
                )
                xqs.append(xq)
            for tm in range(4):
                nc.vector.tensor_copy(xc[:, tm, :], xqs[tm])
                for do in range(DO):
                    nc.sync.dma_start_transpose(
                        xt[:, do, tm * P:(tm + 1) * P],
                        xc[:, tm, do * P:(do + 1) * P],
                    )
            xT[c] = xt

        emit_xprep(0)

        # ---- main pipeline over token chunks ----
        for c in range(NCHUNK):
            if c + 1 < NCHUNK:
                emit_xprep(c + 1)
            xt = xT[c]

            # GEMM1 + bias + gelu -> hT[f-part, fo, tok] (bf16)
            hT = hp.tile([P, FO, TC], BF16, tag="hT")
            for fo in range(FO):
                ps1 = psp.tile([P, TC], F32, tag="ps", name=f"ps1_{c}_{fo}")
                w1t = w1g[fo // 4]
                fi = (fo % 4) * P
                for do in range(DO):
                    nc.tensor.matmul(
                        ps1,
                        w1t[:, do, fi:fi + P],
                        xt[:, do, :],
                        start=(do == 0),
                        stop=(do == DO - 1),
                    )
                nc.scalar.activation(
                    hT[:, fo, :], ps1, GELU, bias=b1sb[:, fo:fo + 1]
                )

            # GEMM2 fo-outer: 8 whole-chunk accumulators (all 8 PSUM
            # banks) so w2 groups are consumed in streaming order.
            accs = [
                psp.tile([P, TC], F32, tag="ps", name=f"acc{c}_{j}")
                for j in range(8)
            ]
            for fo in range(FO):
                w2t = w2g[fo // 4]
                m = fo % 4
                for tt in range(4):
                    lhs = hT[:, fo, tt * P:(tt + 1) * P]
                    for dh in range(2):
                        nc.tensor.matmul(
                            accs[tt * 2 + dh],
                            lhs,
                            w2t[:, m, dh * 512:(dh + 1) * 512],
                            start=(fo == 0),
                            stop=(fo == FO - 1),
                        )
            for j in range(8):
                tt, dh = j // 2, j % 2
                osb = op.tile([P, 512], F32, tag="osb", name=f"osb{c}_{j}")
                nc.vector.tensor_tensor(
                    osb, accs[j], b2sb[:, dh * 512:(dh + 1) * 512],
                    mybir.AluOpType.add,
                )
                row0 = c * TC + tt * P
                nc.scalar.dma_start(
                    out[row0:row0 + P, dh * 512:(dh + 1) * 512], osb
                )


_NC_CACHE = None


def _get_nc():
    global _NC_CACHE
    if _NC_CACHE is None:
        nc = bacc.Bacc(
            "TRN2", target_bir_lowering=False, num_devices=E, num_swdge_queues=4
        )
        x = nc.dram_tensor("x", [TOK, D], F32, kind="ExternalInput").ap()
        w1 = nc.dram_tensor("w1", [D, F], F32, kind="ExternalInput").ap()
        b1 = nc.dram_tensor("b1", [F], F32, kind="ExternalInput").ap()
        w2 = nc.dram_tensor("w2", [F, D], F32, kind="ExternalInput").ap()
        b2 = nc.dram_tensor("b2", [D], F32, kind="ExternalInput").ap()
        out = nc.dram_tensor("out", [TOK, D], F32, kind="ExternalOutput").ap()
        with tile.TileContext(nc) as tctx:
            _build_kernel(tctx, nc, x, w1, b1, w2, b2, out)
        nc.compile()
        _NC_CACHE = nc
    return _NC_CACHE


def kernel(run_opts=None, **inputs):
    x = np.ascontiguousarray(inputs["x"], dtype=np.float32)
    w1 = np.ascontiguousarray(inputs["w1"], dtype=np.float32)
    b1 = np.ascontiguousarray(inputs["b1"], dtype=np.float32)
    w2 = np.ascontiguousarray(inputs["w2"], dtype=np.float32)
    b2 = np.ascontiguousarray(inputs["b2"], dtype=np.float32)

    # x.view(E, B, N, D) in the reference is a pure reshape: expert e owns the
    # contiguous token block e of the flattened (E*B*N, D) buffer.
    xf = x.reshape(E, TOK, D)
    in_maps = [
        {"x": xf[e], "w1": w1[e], "b1": b1[e], "w2": w2[e], "b2": b2[e]}
        for e in range(E)
    ]
    nc = _get_nc()
    res = run_bass_kernel_spmd(
        nc, in_maps, core_ids=list(range(E)), **(run_opts or {})
    )
    outs = np.stack([res.results[e]["out"] for e in range(E)])  # (E, TOK, D)
    if run_opts:
        kernel.last_results = res
    # outputs.view(B, E, N, D) in the reference: reinterpret (E, B*N, D) buffer
    return outs.reshape(B, E, N, D)


# revision 3
# speedup vs baseline: 1.0926x; 1.0926x over previous
"""MoE grouped-GEMM expert MLP for Trainium2, expert-parallel over 8 NeuronCores.

Problem: x:(B=2, E=8, N=2048, D=1024), per-expert 2-layer GELU MLP with
w1:(E, D, F=4096), w2:(E, F, D).  Reference computes
  xe = x.reshape(E, B*N, D)          # pure buffer reinterpretation
  h  = gelu_tanh(xe @ w1 + b1)
  out= h @ w2 + b2                   # reshaped back to (B, E, N, D)

Sharding: expert parallelism - core e runs expert e on its contiguous
token block xe[e] (4096 tokens).  No collectives needed.

Per-core layout: hidden activations kept transposed ("hT" = [f, tok]) so
both weight matrices are consumed in their NATIVE layouts:
  GEMM1: hT[f,tok]  = (w1[d,f] as lhsT).T @ xT[d,tok]
  GEMM2: out[tok,d] = (hT[f,tok] slice as lhsT).T @ w2[f,d]

v3 startup design (evidence: XBAR DMA-transposes serialize the whole
DMA fabric to ~24GB/s while active, so they must never overlap the
weight stream; SWDGE weight delivery is ~280GB/s read-side at best):
 - chunks 0+1: x transposed ON PE (fp32 transpose-mode) from fp32
   HWDGE quarter-loads -> ~28us of PE prefix work with ZERO DMA-fabric
   cost, giving the weight stream a clean 0-120us window.
 - SWDGE carries w1 (g0 split x4, rest half-split) then w2
   (half-split) back-to-back, then the fp32->bf16 cast-to-DRAM DMAs
   for chunks 2-7 (whose XBAR transposes then run in steady state,
   which v1 proved coexists fine).
 - GEMM2 chunk 0 is fo-outer with all 8 PSUM banks as whole-chunk
   accumulators: w2 group k first touched ~6.8k us into GEMM2-c0,
   matching streaming delivery.  Chunks 1-7 use (tt,dh)-outer so
   outputs drain progressively (short kernel tail).
 - outputs + biases on the scalar HWDGE queue; x traffic on sync.

Compute dtype bf16 (fp32 PSUM accumulation), gelu on ScalarE matching
jax.nn.gelu(approximate=True): end-to-end rel-err ~3.4e-3.
"""

import numpy as np

import concourse.bacc as bacc
import concourse.mybir as mybir
import concourse.tile as tile
from concourse.bass_utils import run_bass_kernel_spmd
from concourse.masks import make_identity

E, B, N, D, F = 8, 2, 2048, 1024, 4096
TOK = B * N            # tokens per expert / per core
TC = 512               # token chunk processed per pipeline stage
NCHUNK = TOK // TC     # 8
P = 128
DO = D // P            # 8  d-tiles (GEMM1 contraction)
FO = F // P            # 32 f-tiles (GEMM2 contraction)
FG = 8                 # weight f-groups of 512 (4 f-tiles each)
NPE = 2                # leading chunks transposed on PE (fabric-free)

F32 = mybir.dt.float32
BF16 = mybir.dt.bfloat16
GELU = mybir.ActivationFunctionType.Gelu_apprx_tanh


def _build_kernel(tc_ctx, nc, x, w1, b1, w2, b2, out):
    with (
        tc_ctx.tile_pool(name="wpool", bufs=1) as wp,
        tc_ctx.tile_pool(name="xfp", bufs=3) as xfp,
        tc_ctx.tile_pool(name="xtp", bufs=2) as xtp,
        tc_ctx.tile_pool(name="hpool", bufs=1) as hp,
        tc_ctx.tile_pool(name="opool", bufs=3) as op,
        tc_ctx.tile_pool(name="cpool", bufs=1) as cp,
        tc_ctx.tile_pool(name="dram", bufs=1, space="DRAM") as dp,
        tc_ctx.tile_pool(name="ps", bufs=8, space="PSUM") as psp,
    ):
        # identity for PE-mode fp32 transpose (chunks 0..NPE-1)
        ident = cp.tile([P, P], F32, tag="ident")
        make_identity(nc, ident)

        # ---- biases on the scalar HWDGE queue ----
        b1sb = cp.tile([P, FO], F32, tag="b1")
        nc.scalar.dma_start(b1sb, b1.rearrange("(fo fi) -> fi fo", fi=P))
        b2sb = cp.tile([P, D], F32, tag="b2")
        nc.scalar.dma_start(b2sb, b2[None, :].partition_broadcast(P))

        # ---- weights: SWDGE only, consumption order, fine-grained ----
        # w1 tile (ki, do, fj) = w1[do*128+ki, fg*512+fj] : lhsT for GEMM1
        w1r = w1.rearrange("(do ki) f -> ki do f", ki=P)
        # w2 tile (ki, m, dj) = w2[fg*512 + m*128 + ki, dj] : rhs for GEMM2
        w2r = w2.rearrange("(fg m ki) d -> ki fg m d", ki=P, m=4)
        w1g = [
            wp.tile([P, DO, 512], BF16, tag=f"w1g{fg}", name=f"w1g{fg}")
            for fg in range(FG)
        ]
        w2g = [
            wp.tile([P, 4, D], BF16, tag=f"w2g{fg}", name=f"w2g{fg}")
            for fg in range(FG)
        ]
        for dp_ in range(4):    # first w1 group: 4 sub-DMAs
            nc.gpsimd.dma_start(
                w1g[0][:, 2 * dp_:2 * dp_ + 2, :],
                w1r[:, 2 * dp_:2 * dp_ + 2, 0:512],
            )
        for fg in range(1, FG):  # rest of w1, half-split
            for h in range(2):
                nc.gpsimd.dma_start(
                    w1g[fg][:, 4 * h:4 * h + 4, :],
                    w1r[:, 4 * h:4 * h + 4, fg * 512:(fg + 1) * 512],
                )
        for fg in range(FG):     # w2, half-split
            for h in range(2):
                nc.gpsimd.dma_start(
                    w2g[fg][:, 2 * h:2 * h + 2, :], w2r[:, fg, 2 * h:2 * h + 2]
                )

        # ---- x cast DMAs for chunks NPE..7: AFTER the weights on SWDGE
        # (their XBAR transposes then run clear of the weight stream) ----
        xb = [[None, None] for _ in range(NCHUNK)]
        for c in range(NPE, NCHUNK):
            for half in range(2):
                t = dp.tile([TC, D // 2], BF16, tag=f"xb{c}_{half}")
                nc.gpsimd.dma_start(
                    t,
                    x[c * TC:(c + 1) * TC,
                      half * (D // 2):(half + 1) * (D // 2)],
                )
                xb[c][half] = t

        # fp32 quarter loads for the PE-transposed chunks (sync HWDGE)
        xf32 = []
        for c in range(NPE):
            for tm in range(4):
                xq = xfp.tile([P, D], F32, tag="xq", name=f"xq{c}_{tm}")
                nc.sync.dma_start(
                    xq, x[c * TC + tm * P:c * TC + (tm + 1) * P, :]
                )
                xf32.append(xq)

        xT = [None] * NCHUNK

        def emit_xT(c):
            xt = xtp.tile([P, DO, TC], BF16, tag="xT", name=f"xT{c}")
            if c < NPE:
                # PE-mode fp32 transpose: zero DMA-fabric cost
                for tm in range(4):
                    xq = xf32[c * 4 + tm]
                    for do in range(DO):
                        pt = psp.tile([P, TC], F32, tag="ps",
                                      name=f"psT{c}_{tm}_{do}")
                        nc.tensor.transpose(
                            pt[:, 0:P], xq[:, do * P:(do + 1) * P], ident
                        )
                        nc.vector.tensor_copy(
                            xt[:, do, tm * P:(tm + 1) * P], pt[:, 0:P]
                        )
            else:
                # XBAR DMA transpose from the DRAM bf16 scratch
                for do in range(DO):
                    src = xb[c][do // 4]
                    nc.sync.dma_start_transpose(
                        xt[:, do, :], src[:, (do % 4) * P:(do % 4 + 1) * P]
                    )
            xT[c] = xt

        emit_xT(0)
        emit_xT(1)

        # ---- main pipeline over token chunks ----
        for c in range(NCHUNK):
            if c + 1 >= NPE and c + 1 < NCHUNK:
                emit_xT(c + 1)
            xt = xT[c]

            # GEMM1 + bias + gelu -> hT[f-part, fo, tok] (bf16)
            hT = hp.tile([P, FO, TC], BF16, tag="hT")
            for fo in range(FO):
                ps1 = psp.tile([P, TC], F32, tag="ps", name=f"ps1_{c}_{fo}")
                w1t = w1g[fo // 4]
                fi = (fo % 4) * P
                for do in range(DO):
                    nc.tensor.matmul(
                        ps1,
                        w1t[:, do, fi:fi + P],
                        xt[:, do, :],
                        start=(do == 0),
                        stop=(do == DO - 1),
                    )
                nc.scalar.activation(
                    hT[:, fo, :], ps1, GELU, bias=b1sb[:, fo:fo + 1]
                )

            if c == 0:
                # fo-outer GEMM2: 8 whole-chunk accumulators; w2 group k
                # first touched ~6.8k us in -> matches streaming delivery
                accs = [
                    psp.tile([P, TC], F32, tag="ps", name=f"acc{c}_{j}")
                    for j in range(8)
                ]
                for fo in range(FO):
                    w2t = w2g[fo // 4]
                    m = fo % 4
                    for tt in range(4):
                        lhs = hT[:, fo, tt * P:(tt + 1) * P]
                        for dh in range(2):
                            nc.tensor.matmul(
                                accs[tt * 2 + dh],
                                lhs,
                                w2t[:, m, dh * 512:(dh + 1) * 512],
                                start=(fo == 0),
                                stop=(fo == FO - 1),
                            )
                for j in range(8):
                    tt, dh = j // 2, j % 2
                    osb = op.tile([P, 512], F32, tag="osb", name=f"osb{c}_{j}")
                    nc.vector.tensor_tensor(
                        osb, accs[j], b2sb[:, dh * 512:(dh + 1) * 512],
                        mybir.AluOpType.add,
                    )
                    row0 = c * TC + tt * P
                    nc.scalar.dma_start(
                        out[row0:row0 + P, dh * 512:(dh + 1) * 512], osb
                    )
            else:
                # (tt,dh)-outer GEMM2: outputs drain progressively
                for tt in range(4):
                    for dh in range(2):
                        ps2 = psp.tile([P, TC], F32, tag="ps",
                                       name=f"ps2_{c}_{tt}_{dh}")
                        for fo in range(FO):
                            nc.tensor.matmul(
                                ps2,
                                hT[:, fo, tt * P:(tt + 1) * P],
                                w2g[fo // 4][:, fo % 4,
                                             dh * 512:(dh + 1) * 512],
                                start=(fo == 0),
                                stop=(fo == FO - 1),
                            )
                        osb = op.tile([P, 512], F32, tag="osb",
                                      name=f"osb{c}_{tt}_{dh}")
                        nc.vector.tensor_tensor(
                            osb, ps2, b2sb[:, dh * 512:(dh + 1) * 512],
                            mybir.AluOpType.add,
                        )
                        row0 = c * TC + tt * P
                        nc.scalar.dma_start(
                            out[row0:row0 + P, dh * 512:(dh + 1) * 512], osb
                        )


_NC_CACHE = None


def _get_nc():
    global _NC_CACHE
    if _NC_CACHE is None:
        nc = bacc.Bacc(
            "TRN2", target_bir_lowering=False, num_devices=E, num_swdge_queues=4
        )
        x = nc.dram_tensor("x", [TOK, D], F32, kind="ExternalInput").ap()
        w1 = nc.dram_tensor("w1", [D, F], F32, kind="ExternalInput").ap()
        b1 = nc.dram_tensor("b1", [F], F32, kind="ExternalInput").ap()
        w2 = nc.dram_tensor("w2", [F, D], F32, kind="ExternalInput").ap()
        b2 = nc.dram_tensor("b2", [D], F32, kind="ExternalInput").ap()
        out = nc.dram_tensor("out", [TOK, D], F32, kind="ExternalOutput").ap()
        with tile.TileContext(nc) as tctx:
            _build_kernel(tctx, nc, x, w1, b1, w2, b2, out)
        nc.compile()
        _NC_CACHE = nc
    return _NC_CACHE


def kernel(run_opts=None, **inputs):
    x = np.ascontiguousarray(inputs["x"], dtype=np.float32)
    w1 = np.ascontiguousarray(inputs["w1"], dtype=np.float32)
    b1 = np.ascontiguousarray(inputs["b1"], dtype=np.float32)
    w2 = np.ascontiguousarray(inputs["w2"], dtype=np.float32)
    b2 = np.ascontiguousarray(inputs["b2"], dtype=np.float32)

    # x.view(E, B, N, D) in the reference is a pure reshape: expert e owns the
    # contiguous token block e of the flattened (E*B*N, D) buffer.
    xf = x.reshape(E, TOK, D)
    in_maps = [
        {"x": xf[e], "w1": w1[e], "b1": b1[e], "w2": w2[e], "b2": b2[e]}
        for e in range(E)
    ]
    nc = _get_nc()
    res = run_bass_kernel_spmd(
        nc, in_maps, core_ids=list(range(E)), **(run_opts or {})
    )
    outs = np.stack([res.results[e]["out"] for e in range(E)])  # (E, TOK, D)
    if run_opts:
        kernel.last_results = res
    # outputs.view(B, E, N, D) in the reference: reinterpret (E, B*N, D) buffer
    return outs.reshape(B, E, N, D)
